# revision 1
# baseline (speedup 1.0000x reference)
"""Bicubic 4x downsample (MATLAB imresize-style) on Trainium2, 8-core data parallel.

Math: the reference is a separable resize: H-resize then W-resize, each a
gather + weighted sum along one axis. Both are linear maps, so per channel
image X [H, W]:

    out = WH @ X @ WW^T,   WH [OH, H] (banded),  WW [OW, W] (banded)

We build the dense banded matrices host-side from (w_h, idx_h, w_w, idx_w)
(boundary reflection folds in for free) and evaluate both contractions on
the PE array, using the image as the *stationary* (weights) operand so each
image element streams through the PE exactly once:

  stage 1:  out1T[w, oh] = sum_h  X[h, w]    * WHT[h, oh]   (lhsT = X tile)
  stage 2:  out2 [oh,ow] = sum_w  out1T[w,oh]* WWT[w, ow]   (lhsT = out1T)

Both stages exploit the band structure: an h-tile of 128 input rows only
contributes to a ~35-wide window of output rows, so the moving operand is a
narrow slice of the packed weight matrix. PSUM per-element has_written bits
accumulate overlapping windows across tiles.

Precision/perf: all PE operands are bf16 (fp32 matmuls stream at 4 cycles
per row on TRN2; bf16 at 1). The fp32->bf16 input cast rides the load DMA
for free (SWDGE cast path, nc.gpsimd), weights are cast host-side, and
PSUM accumulation stays fp32; measured absmax-rel error ~4.4e-3 vs the
fp32 reference. Loads stay at h-tile granularity (9 SWDGE DMAs/channel)
so stage-1 chains track the load wave, with 3 channel buffers in flight;
stage-2 runs bank-major over PSUM so the PE never drains between
accumulation-group switches; outputs are copied f32 via ACT and stored on
the two HWDGE queues.

Sharding: pure data parallel, batch b -> core b (8 batches, 8 cores).
"""

import numpy as np

TILE = 128


def _ensure_concourse():
    try:
        import concourse  # noqa: F401
    except ImportError:
        import sys
        for p in ("/opt/trn_rl_repo", "/root/.axon_site/_ro/trn_rl_repo"):
            if p not in sys.path:
                sys.path.insert(0, p)


_PATCHED = False


def _patch_tile_drain():
    """This walrus build rejects >1 sem wait on TPB_CTRL instructions (the
    Tile exit Drain). Split the final drain's waits into single-wait nops."""
    global _PATCHED
    if _PATCHED:
        return
    from concourse import tile
    from concourse.vector_clock import VectorClock, ScopedClock

    def _drain_and_barrier(self, tick_clock, wait_clock):
        gc = tick_clock.global_clock
        n = len(gc)
        for i in range(n):
            if gc[i] <= 0:
                continue
            vc = VectorClock([gc[j] if j == i else 0 for j in range(n)])
            nop_inst = self.nc.sync.nop(nofuse=True, hint="drain_split")
            wait_clock.add_sem_waits(nop_inst.ins, ScopedClock({None: vc}))
        self.nc.sync.drain()
        self.nc.all_engine_barrier()
        assert self.sems is not None
        popped = self.nc._tile_sem_poison_stack.pop()
        assert popped is self._sem_poison
        self.nc.clear_and_free_semaphores(list(self.sems.allocated().values()))
        self.nc.all_engine_barrier()

    tile.TileContext._drain_and_barrier = _drain_and_barrier
    _PATCHED = True


def _split_multi_waits(nc):
    """This walrus build rejects instructions carrying >1 sem wait. Hoist all
    but the last wait of any instruction onto same-engine nops placed
    immediately before it (engine streams execute block order in-order, so
    waiting on a preceding nop is equivalent)."""
    from concourse import mybir

    uid = 0
    for fn in nc.m.functions:
        for bb in fn.blocks:
            insts = bb.instructions  # live list
            new_list = []
            changed = False
            for ins in list(insts):
                si = ins.sync_info
                if si is not None and len(si.on_wait) > 1:
                    waits = list(si.on_wait)
                    for wt in waits[:-1]:
                        uid += 1
                        nop = mybir.InstNoOp(
                            name=f"ws_nop_{uid}",
                            engine=ins.engine,
                            ins=[],
                            outs=[],
                            sync_info=mybir.SyncInfo(on_wait=[wt], on_update=[]),
                            bass_nofuse=True,
                        )
                        new_list.append(nop)
                    ins.sync_info = mybir.SyncInfo(
                        on_wait=[waits[-1]], on_update=list(si.on_update)
                    )
                    changed = True
                new_list.append(ins)
            if changed:
                insts.clear()
                insts.extend(new_list)


def _dedup_ldweights(nc):
    """Drop InstLdweights that reload the exact weights already resident in
    the PE array (same AP, no other LDW between). Matmuls don't disturb the
    array, and PE executes its stream in order, so this is safe; any sem
    waits on a dropped LDW move to the next PE instruction."""
    from concourse import mybir

    n_drop = 0
    for fn in nc.m.functions:
        for bb in fn.blocks:
            insts = bb.instructions
            new_list = []
            last_ldw_key = None
            pending_waits = []
            for ins in list(insts):
                if ins.engine != mybir.EngineType.PE:
                    new_list.append(ins)
                    continue
                if isinstance(ins, mybir.InstLdweights):
                    ap = ins.ins[0]
                    key = (str(ap), str(getattr(ins, "is_transpose", None)),
                           str(getattr(ins, "tile_size", None)),
                           str(getattr(ins, "tile_position", None)))
                    if key == last_ldw_key:
                        si = ins.sync_info
                        if si is not None and si.on_wait:
                            pending_waits.extend(si.on_wait)
                        assert not (si is not None and si.on_update), \
                            "dropped LDW carries sem update"
                        n_drop += 1
                        continue
                    last_ldw_key = key
                elif not isinstance(ins, (mybir.InstMatmult, mybir.InstNoOp)):
                    # unknown PE instruction: be conservative
                    last_ldw_key = None
                if pending_waits:
                    si = ins.sync_info
                    cur = list(si.on_wait) if si is not None else []
                    upd = list(si.on_update) if si is not None else []
                    ins.sync_info = mybir.SyncInfo(
                        on_wait=pending_waits + cur, on_update=upd)
                    pending_waits = []
                new_list.append(ins)
            assert not pending_waits
            insts.clear()
            insts.extend(new_list)
    return n_drop


def _dense_t(weights, indices, in_len):
    """Dense transposed resize matrix [in_len, out_len]:
    M[i, o] = sum over taps p with indices[o, p] == i of weights[o, p]."""
    w = np.asarray(weights, np.float32)
    idx = np.asarray(indices, np.int64)
    out_len, ntap = w.shape
    m = np.zeros((in_len, out_len), np.float32)
    ocol = np.repeat(np.arange(out_len), ntap)
    np.add.at(m, (idx.ravel(), ocol), w.ravel())
    return m


def _windows(mat_t):
    """Per 128-row tile of the [in, out] matrix: (out_lo, out_hi, packed_off).
    Band structure makes the nonzero columns of each tile contiguous-ish;
    we take the [first, last+1] span (interior zeros just add zeros)."""
    wins = []
    off = 0
    for t0 in range(0, mat_t.shape[0], TILE):
        blk = mat_t[t0:t0 + TILE]
        nz = np.flatnonzero(np.any(blk != 0.0, axis=0))
        lo, hi = int(nz[0]), int(nz[-1]) + 1
        wins.append((lo, hi, off))
        off += hi - lo
    return wins, off


def _pack(mat_t, wins):
    total = wins[-1][2] + (wins[-1][1] - wins[-1][0])
    p = np.zeros((TILE, total), np.float32)
    for (lo, hi, off), t0 in zip(wins, range(0, mat_t.shape[0], TILE)):
        blk = mat_t[t0:t0 + TILE, lo:hi]
        p[:blk.shape[0], off:off + (hi - lo)] = blk
    return p


def _oh_chunks(n):
    return [(a, min(a + TILE, n)) for a in range(0, n, TILE)]


def _build_program(C, H, W, OH, OW, hwins, wwins, W1, W2, repeat=1, mode="full", ps1_bufs=2, ps2_mult=2, cast=True, xbufs=2, bigdma=False, dedup=False, o1bufs=17, osbufs=2, wsplit=1):
    from concourse import bass, tile, mybir

    f32 = mybir.dt.float32
    # compute dtype: bf16 operands make matmuls 4x faster on the PE (fp32
    # streams at 4 cycles/row + slow weight load) and halve SBUF footprint.
    # The fp32->bf16 cast rides the load DMA for free (SWDGE cast path).
    cdt = mybir.dt.bfloat16 if cast else f32
    nc = bass.Bass()
    x_d = nc.declare_dram_parameter("x", [C, H, W], f32, isOutput=False)
    wht_d = nc.declare_dram_parameter("wht", [TILE, W1], cdt, isOutput=False)
    wwt_d = nc.declare_dram_parameter("wwt", [TILE, W2], cdt, isOutput=False)
    out_d = nc.declare_dram_parameter("out", [C, OH, OW], f32, isOutput=True)

    HT = (H + TILE - 1) // TILE
    WT = (W + TILE - 1) // TILE
    ohc = _oh_chunks(OH)

    with tile.TileContext(nc) as tc:
        with (
            tc.tile_pool(name="consts", bufs=1) as cpool,
            tc.tile_pool(name="xch", bufs=xbufs) as xpool,
            tc.tile_pool(name="o1", bufs=o1bufs) as o1pool,
            tc.tile_pool(name="osb", bufs=osbufs) as opool,
            tc.tile_pool(name="ps1", bufs=ps1_bufs, space=bass.MemorySpace.PSUM) as ps1pool,
            tc.tile_pool(name="ps2", bufs=ps2_mult * len(ohc), space=bass.MemorySpace.PSUM) as ps2pool,
        ):
            wht_sb = cpool.tile([TILE, W1], cdt)
            nc.sync.dma_start(out=wht_sb[:, :], in_=wht_d[:, :])
            wwt_sb = cpool.tile([TILE, W2], cdt)
            nc.scalar.dma_start(out=wwt_sb[:, :], in_=wwt_d[:, :])

            o1_dummy = None
            if mode == "nocopy":
                o1_dummy = cpool.tile([TILE, OH], cdt, name="o1_dummy")
                nc.gpsimd.memset(o1_dummy[:, :], 0.0)
            xcc = None
            if mode in ("comp", "s1c", "s1na", "s1ws", "s1t2", "s1t4"):
                # loads hoisted out of the body loop: measures pure
                # compute-side throughput (stale data, timing-realistic)
                xcc = cpool.tile([TILE, HT * W], cdt, name="xcc")
                for ht in range(HT):
                    p = min(TILE, H - TILE * ht)
                    nc.gpsimd.dma_start(
                        out=xcc[0:p, ht * W:ht * W + W],
                        in_=x_d[0, TILE * ht:TILE * ht + p, :],
                    )
            for rc in range(repeat * C):
                c = rc % C
                # whole channel resident: [128, HT*W], h-tile ht at free
                # offset ht*W (row-major rows are contiguous in DRAM)
                if xcc is not None:
                    xc = xcc
                else:
                    xc = xpool.tile([TILE, HT * W], cdt)
                if xcc is not None:
                    pass
                elif bigdma and cast:
                    # two cast DMAs per channel: full 128-row tiles as one
                    # 3D-AP transfer [p, t, w], plus the 56-row tail
                    tf = H // TILE  # full tiles
                    nc.gpsimd.dma_start(
                        out=xc[0:TILE, 0:tf * W].rearrange(
                            "p (t w) -> p t w", t=tf),
                        in_=x_d[c, 0:tf * TILE, :].rearrange(
                            "(t p) w -> p t w", p=TILE),
                    )
                    pr = H - tf * TILE
                    if pr:
                        nc.gpsimd.dma_start(
                            out=xc[0:pr, tf * W:tf * W + W],
                            in_=x_d[c, tf * TILE:H, :],
                        )
                else:
                    for ht in range(HT):
                        p = min(TILE, H - TILE * ht)
                        if cast:
                            # SWDGE cast-DMA: fp32 HBM -> bf16 SBUF inline
                            eng = nc.gpsimd
                        else:
                            eng = nc.sync if (mode == "dsp" or ht % 2 == 0) else nc.scalar
                        wc = W // wsplit
                        for j in range(wsplit):
                            # w-split halves the load->matmul handoff
                            # granularity (byte-range dep tracking unblocks
                            # left w-tiles while the right half lands)
                            eng.dma_start(
                                out=xc[0:p, ht * W + j * wc:ht * W + (j + 1) * wc],
                                in_=x_d[c, TILE * ht:TILE * ht + p, j * wc:(j + 1) * wc],
                            )

                if mode == "dma":
                    for k, (a, b) in enumerate(ohc):
                        osb = opool.tile([TILE, OW], f32)
                        nc.vector.tensor_copy(osb[0:b - a, :], xc[0:b - a, 0:OW])
                        eng = nc.sync if k % 2 == 0 else nc.scalar
                        eng.dma_start(out=out_d[c, a:b, :], in_=osb[0:b - a, :])
                    continue
                if mode == "s1ws":
                    # timing-only: stage-1 with weights stationary (lhsT) and
                    # the image as the wide moving operand (N=480). Results
                    # land transposed and are NOT accumulated correctly.
                    NW = 4
                    for ht in range(HT):
                        p = min(TILE, H - TILE * ht)
                        lo, hi, off = hwins[ht]
                        win = hi - lo
                        psw = ps1pool.tile([TILE, OW], f32)
                        for j in range(NW):
                            nc.tensor.matmul(
                                psw[0:win, 0:OW],
                                wht_sb[0:p, off:off + win],
                                xc[0:p, ht * W + j * OW:ht * W + (j + 1) * OW],
                                start=True,
                                stop=True,
                            )
                    for k, (a, b) in enumerate(ohc):
                        osb = opool.tile([TILE, OW], f32)
                        nc.vector.tensor_copy(osb[0:b - a, :], xc[0:b - a, 0:OW])
                        eng = nc.sync if k % 2 == 0 else nc.scalar
                        eng.dma_start(out=out_d[c, a:b, :], in_=osb[0:b - a, :])
                    continue
                ps2s = [ps2pool.tile([TILE, OW], f32, name="ps2", tag="ps2") for _ in ohc]

                def s2_one(wt, o1, pw):
                    wlo, whi, woff = wwins[wt]
                    for k, (a, b) in enumerate(ohc):
                        nc.tensor.matmul(
                            ps2s[k][0:b - a, wlo:whi],
                            o1[0:pw, a:b],
                            wwt_sb[0:pw, woff:woff + (whi - wlo)],
                            start=(wt == 0),
                            stop=(wt == WT - 1),
                        )

                # stage 1 for the whole channel; buffer all o1 tiles
                o1s = []
                for wt in range(WT):
                    pw = min(TILE, W - TILE * wt)
                    ps1 = ps1pool.tile([TILE, OH], f32)
                    # stage 1: out1T[w, oh] += X[h, w] * WHT[h, oh]
                    sub = {"s1t2": 64, "s1t4": 32}.get(mode, 0)
                    for ht in range(HT):
                        p = min(TILE, H - TILE * ht)
                        lo, hi, off = hwins[ht]
                        if sub:
                            # column-tiled: sub-MMs land on independent PE
                            # tiles (out partition base picks the tile), so
                            # fill/drain of neighbors can overlap
                            for mo in range(0, pw, sub):
                                m2 = min(sub, pw - mo)
                                base = ht * W + TILE * wt + mo
                                nc.tensor.matmul(
                                    ps1[mo:mo + m2, lo:hi],
                                    xc[0:p, base:base + m2],
                                    wht_sb[0:p, off:off + (hi - lo)],
                                    start=(ht == 0),
                                    stop=(ht == HT - 1),
                                )
                            continue
                        nc.tensor.matmul(
                            ps1[0:pw, lo:hi],
                            xc[0:p, ht * W + TILE * wt:ht * W + TILE * wt + pw],
                            wht_sb[0:p, off:off + (hi - lo)],
                            start=True if mode == "s1na" else (ht == 0),
                            stop=True if mode == "s1na" else (ht == HT - 1),
                        )
                    if mode in ("s1", "s1c", "s1na", "s1t2", "s1t4"):
                        continue
                    if mode == "nocopy":
                        o1 = o1_dummy
                    else:
                        # cast-copy PSUM f32 -> SBUF bf16 feeds stage 2
                        o1 = o1pool.tile([TILE, OH], cdt)
                        if mode == "cpalt" and wt % 2 == 1:
                            nc.scalar.copy(o1[0:pw, :], ps1[0:pw, :])
                        else:
                            nc.vector.tensor_copy(o1[0:pw, :], ps1[0:pw, :])
                    o1s.append((o1, pw))
                    if mode == "s2il" and len(o1s) >= 2:
                        s2_one(wt - 1, *o1s[wt - 1])
                if mode == "s2il":
                    s2_one(WT - 1, *o1s[WT - 1])
                # stage 2: out2[oh, ow] += out1T[w, oh] * WWT[w, ow]
                # bank-major order: all 15 w-tiles of one PSUM bank back-to-back
                # so PE never drains between accumulation-group switches
                if mode not in ("s1", "s1c", "s1na", "s1t2", "s1t4", "nos2", "s2il"):
                    for k, (a, b) in enumerate(ohc):
                        for wt in range(WT):
                            o1, pw = o1s[wt]
                            wlo, whi, woff = wwins[wt]
                            nc.tensor.matmul(
                                ps2s[k][0:b - a, wlo:whi],
                                o1[0:pw, a:b],
                                wwt_sb[0:pw, woff:woff + (whi - wlo)],
                                start=(wt == 0),
                                stop=(wt == WT - 1),
                            )
                for k, (a, b) in enumerate(ohc):
                    osb = opool.tile([TILE, OW], f32)
                    if mode in ("s1", "s1c", "s1na", "s1t2", "s1t4", "nos2"):
                        nc.vector.tensor_copy(osb[0:b - a, :], xc[0:b - a, 0:OW])
                    elif mode == "odve":
                        nc.vector.tensor_copy(osb[0:b - a, :], ps2s[k][0:b - a, :])
                    else:
                        # ACT for output copies: frees DVE for the o1 chain and
                        # unblocks ps2 bank reuse sooner (~100us/body measured)
                        nc.scalar.copy(osb[0:b - a, :], ps2s[k][0:b - a, :])
                    eng = nc.sync if (mode == "osp" or k % 2 == 0) else nc.scalar
                    eng.dma_start(out=out_d[c, a:b, :], in_=osb[0:b - a, :])

    if dedup:
        n = _dedup_ldweights(nc)
        import logging; logging.getLogger(__name__).info(f"dedup_ldweights dropped {n}")
    _split_multi_waits(nc)
    return nc


def _as_bf16(a):
    import ml_dtypes
    return np.asarray(a, np.float32).astype(ml_dtypes.bfloat16)


def kernel(x, w_h, idx_h, w_w, idx_w, _trace=False):
    _ensure_concourse()
    _patch_tile_drain()
    from concourse.bass_utils import run_bass_kernel_spmd

    x = np.ascontiguousarray(np.asarray(x, np.float32))
    B, C, H, W = x.shape
    wht_t = _dense_t(w_h, idx_h, H)   # [H, OH]
    wwt_t = _dense_t(w_w, idx_w, W)   # [W, OW]
    OH, OW = wht_t.shape[1], wwt_t.shape[1]

    hwins, W1 = _windows(wht_t)
    wwins, W2 = _windows(wwt_t)
    wht_packed = _as_bf16(_pack(wht_t, hwins))
    wwt_packed = _as_bf16(_pack(wwt_t, wwins))

    nc = _build_program(C, H, W, OH, OW, hwins, wwins, W1, W2, xbufs=3)

    in_maps = [
        {"x": x[b], "wht": wht_packed, "wwt": wwt_packed} for b in range(B)
    ]
    res = run_bass_kernel_spmd(nc, in_maps, list(range(B)), trace=bool(_trace))
    out = np.stack([res.results[i]["out"] for i in range(B)], axis=0)
    if _trace:
        return out, res
    return out

